# revision 43
# baseline (speedup 1.0000x reference)
"""Trainium2 Bass kernel for nn_Conv2dShareQ (vq_codebook) — Winograd F(2,3).

Computation (see reference):
    wq = centroids[labels]            # [512, 256, 3, 3] fp32, 16-entry codebook
    out0 = conv2d(x, wq[:256], bias[0])   # NCHW, 3x3, stride 1, pad 1
    out1 = conv2d(x, wq[256:], bias[1])
    return (out0, out1)

Sharding across 8 NeuronCores: 4-way data-parallel over batch x 2-way over the
two weight-sharing convs.  Core c handles images [4b, 4b+4) with b = c // 2 and
conv group g = c % 2 (256 output channels).  No collectives.

Per-core kernel: Winograd F(2,3) applied along H (m=2 outputs per tile row),
direct convolution along W.  This cuts PE work to 2/3 of direct conv:
  V planes (input transform, bf16, all unit-stride):
      x is DMA'd row-deinterleaved into parity-split padded buffers
      xe = padded even rows {0,2,..,56}, xo = padded odd rows {1,3,..,57},
      so d_r (= padded rows 2ty+r) become contiguous views:
      d0=xe[0:28], d1=xo[0:28], d2=xe[1:29], d3=xo[1:29]
      V0=d0-d2, V1=d1+d2, V2=d2-d1, V3=d1-d3
  U planes (weight transform; 0.5 factors folded into output transform):
      U0=g0, U1'=g0+g1+g2, U2'=g0-g1+g2, U3=g2     (g_ky from codebook gather)
  M'_p[o,ty,x] = sum_{c,kx} U'_p[o,c,kx] V_p[c,ty,x+kx]   (PE, 6 matmuls/point)
  output (ACT/DVE/GPSIMD, fp32):
      out[2ty+0] = M0 + 0.5(M1'+M2') + b;  out[2ty+1] = 0.5(M1'-M2') - M3 + b

Codebook gather fully on DVE (GPSIMD tensor_scalar is ~12x slower, and
concurrent GPSIMD elementwise work contends with DVE for SBUF, slowing DVE
ops up to 6x): t_v = (labels == v) * centroids[v] via one tensor_scalar
(is_equal, mult), then a 4-level add tree.  The PE idles during the
gather (a warm-up stream would steal SBUF bandwidth from the DVE) and a
late ~7.5us dummy-matmul burst — launched via a data dependency on the
last eq pass — re-warms the HAM clock right before the real matmuls.
DMAs are spread across the sync/scalar/gpsimd engine queues (a single
queue caps at ~126 GB/s).
"""

import sys

for _p in ("/opt/trn_rl_repo", "/root/.axon_site/_ro/trn_rl_repo"):
    if _p not in sys.path:
        sys.path.append(_p)

import numpy as np

import concourse.bass as bass
import concourse.mybir as mybir
from concourse.tile import TileContext, ScopedClock
from concourse.tile_scheduler import N_PROCS
from bass_rust import VectorClock
from concourse.bass_utils import run_bass_kernel_spmd

F32 = mybir.dt.float32
BF16 = mybir.dt.bfloat16
I32 = mybir.dt.int32

N_IMG_PER_CORE = 4      # 16 images / 4 batch shards
N_KT = 2                # 256 input channels / 128
N_MT = 2                # 256 output channels per conv group / 128
H = W = 56
WP = 58                 # padded width
HPAR = 29               # rows per parity buffer (28 data + 1 pad row)
HW = H * W              # 3136
N_CENT = 16
CH = 9 * 128            # 1152 free elems per (mt, kt) label chunk
LAB_FREE = N_MT * CH    # 2304 free elems/partition per k-tile
N_TY = 28               # Winograd tile rows (2 output rows each)
TY_T = 7                # ty rows per psum tile
N_T4 = N_TY // TY_T     # 4 psum tile groups per (im, mt)
NFREE = TY_T * W        # 392 columns per point-psum

# conv pair order: three mt=0 pairs of runway before the first mt=1 pair
# (the mt=1 gather shares DVE with V transforms and PSUM evictions), with a
# 3-image-deep V ring
PAIR_ORDER = [(0, 0), (1, 0), (2, 0), (0, 1), (1, 1), (3, 0), (2, 1), (3, 1)]


class SplitDrainTileContext(TileContext):
    """Tail drain split one proc per drain: this walrus build rejects CTRL
    instructions carrying more than one sem wait."""

    def _drain_and_barrier(self, tick_clock, wait_clock):
        gc = tick_clock.global_clock
        for p in range(N_PROCS):
            t = gc[p]
            if t <= 0:
                continue
            vec = [t if q == p else 0 for q in range(N_PROCS)]
            d = self.nc.sync.drain()
            wait_clock.add_sem_waits(d.ins, ScopedClock({None: VectorClock(vec)}))
        self.nc.all_engine_barrier()
        assert self.sems is not None
        popped = self.nc._tile_sem_poison_stack.pop()
        assert popped is self._sem_poison
        self.nc.clear_and_free_semaphores(list(self.sems.allocated().values()))
        self.nc.all_engine_barrier()


def _split_multi_waits(nc, limit=1):
    """This walrus build rejects instructions carrying more than one sem wait
    ("Too many sync wait commands").  Hoist excess waits onto wait-only
    EventSemaphore instructions inserted just before, on the same engine."""
    for f in nc.m.functions:
        for bb in f.blocks:
            out = []
            for ins in bb.instructions:
                si = ins.sync_info
                if si is not None and si.on_wait and len(si.on_wait) > limit:
                    waits = list(si.on_wait)
                    for w in waits[:-limit]:
                        es = mybir.InstEventSemaphore(
                            name=f"waitsplit_{nc.next_id()}", ins=[], outs=[])
                        es.engine = ins.engine
                        es.sync_info = mybir.SyncInfo(on_wait=[w], on_update=[])
                        out.append(es)
                    si.on_wait = waits[-limit:]
                out.append(ins)
            bb.instructions[:] = out


def build_program():
    nc = bass.Bass()

    x_in = nc.dram_tensor("x", [N_IMG_PER_CORE, N_KT, 128, HW], F32,
                          kind="ExternalInput")
    labels_in = nc.dram_tensor("labels", [N_KT, 128, LAB_FREE], I32,
                               kind="ExternalInput")
    cent_in = nc.dram_tensor("centroids", [N_CENT], F32, kind="ExternalInput")
    bias_in = nc.dram_tensor("bias", [N_MT, 128], F32, kind="ExternalInput")
    out = nc.dram_tensor("out", [N_IMG_PER_CORE, N_MT, 128, HW], F32,
                         kind="ExternalOutput")

    N_WARM = 40   # late HAM warm-up burst (~9us) before the real matmuls

    with SplitDrainTileContext(nc) as tc:
        with (
            tc.tile_pool(name="consts", bufs=1) as consts,
            tc.tile_pool(name="lab_f", bufs=1) as lab_f_pool,
            tc.tile_pool(name="wq", bufs=1) as wq_pool,
            tc.tile_pool(name="u12", bufs=1) as u12_pool,
            tc.tile_pool(name="stmp", bufs=1) as stmp_pool,
            tc.tile_pool(name="tbuf", bufs=1) as tbuf_pool,
            tc.tile_pool(name="xpad", bufs=1) as xpad_pool,
            tc.tile_pool(name="vbuf", bufs=3) as vbuf_pool,
            tc.tile_pool(name="lstage", bufs=2) as lstage_pool,
            tc.tile_pool(name="xstage", bufs=1) as xstage_pool,
            tc.tile_pool(name="obuf", bufs=2) as obuf_pool,
            tc.tile_pool(name="mh", bufs=3) as mh_pool,
            tc.tile_pool(name="asd", bufs=2) as asd_pool,
            tc.tile_pool(name="psum", bufs=8, space="PSUM") as psum_pool,
        ):
            lab_f = [lab_f_pool.tile([128, LAB_FREE], BF16, tag=f"lf{kt}",
                                     name=f"lab_f{kt}")
                     for kt in range(N_KT)]
            lab_stage = {}

            def load_labels(mt):
                for kt in range(N_KT):
                    sl = slice(mt * CH, (mt + 1) * CH)
                    li = lstage_pool.tile([128, CH], I32, tag="ls",
                                          name=f"lab_st{mt}_{kt}")
                    if mt == 0:
                        # startup labels quartered across three queues so the
                        # first eq pass can start ~5us in
                        e1 = nc.scalar if kt == 0 else nc.gpsimd
                        e2 = nc.sync if kt == 0 else nc.scalar
                    else:
                        e1 = e2 = nc.sync
                    e1.dma_start(out=li[0:64, :], in_=labels_in[kt][0:64, sl])
                    e2.dma_start(out=li[64:128, :], in_=labels_in[kt][64:128, sl])
                    lab_stage[(mt, kt)] = li

            # ---- tiny const DMAs (centroids gate the eq ops — first) ----
            cent_sb = consts.tile([128, N_CENT], F32)
            cent_bcast = bass.AP(tensor=cent_in[:].tensor, offset=0,
                                 ap=[[0, 128], [1, N_CENT]])
            nc.sync.dma_start(out=cent_sb[:], in_=cent_bcast)

            # ---- HAM warm-up staging buffer (burst emitted later: a PE
            # that idles during the gather does not steal SBUF bandwidth
            # from the DVE eq/tree ops) ----
            warm_sb = consts.tile([128, 512], BF16)
            nc.gpsimd.memset(warm_sb[:], 0.0)

            # ---- x staging: row-parity split, padded [128, 29, 58] bf16 ----
            # xe rows = padded {0,2,..,56}: row0 = zero pad, rows 1:29 = orig
            # odd rows 1,3,..,55.  xo rows = padded {1,3,..,57}: rows 0:28 =
            # orig even rows 0,2,..,54, row 28 = zero pad.
            xe = [xpad_pool.tile([128, HPAR, WP], BF16, tag=f"xe{kt}",
                                 name=f"xe{kt}") for kt in range(N_KT)]
            xo = [xpad_pool.tile([128, HPAR, WP], BF16, tag=f"xo{kt}",
                                 name=f"xo{kt}") for kt in range(N_KT)]
            for kt in range(N_KT):
                # borders zeroed once; interior copies never touch them
                nc.gpsimd.memset(xe[kt][:, 0, :], 0.0)
                nc.gpsimd.memset(xe[kt][:, :, 0:1], 0.0)
                nc.gpsimd.memset(xe[kt][:, :, WP - 1:WP], 0.0)
                nc.gpsimd.memset(xo[kt][:, HPAR - 1, :], 0.0)
                nc.gpsimd.memset(xo[kt][:, :, 0:1], 0.0)
                nc.gpsimd.memset(xo[kt][:, :, WP - 1:WP], 0.0)

            def load_x(im, kt):
                # one contiguous DMA (3136B rows = efficient descriptors; the
                # row-parity split rides on the ACT copies' strided input).
                # x DMAs alternate between the sync and gpsimd engine queues:
                # all DMAs on one queue cap at ~126 GB/s.
                xs = xstage_pool.tile([128, HW], F32, tag="xs",
                                      name=f"xs{im}_{kt}")
                # scalar dispatches follow the pad-copies on the same
                # engine, so the xstage WAR wait never blocks the queue
                eng = nc.scalar if kt == 0 else nc.gpsimd
                eng.dma_start(out=xs[:], in_=x_in[im, kt])
                xv = xs[:].rearrange("p (h w) -> p h w", h=H)
                # pad-copies on ACT (fast dtype-cast engine; GPSIMD casts are
                # ~5x slower and DVE is the busiest engine)
                nc.scalar.activation(
                    out=xe[kt][:, 1:HPAR, 1:W + 1], in_=xv[:, 1:56:2, :],
                    func=mybir.ActivationFunctionType.Copy, scale=1.0)
                nc.scalar.activation(
                    out=xo[kt][:, 0:HPAR - 1, 1:W + 1], in_=xv[:, 0:55:2, :],
                    func=mybir.ActivationFunctionType.Copy, scale=1.0)

            # ---- Winograd input transform: V planes (unit-stride ops) ----
            vtiles = {}

            def v_transform(im, kt):
                vt = vbuf_pool.tile([128, 4, N_TY, WP], BF16, tag=f"v{kt}",
                                    name=f"v{im}_{kt}")
                d0 = xe[kt][:, 0:N_TY, :]
                d1 = xo[kt][:, 0:N_TY, :]
                d2 = xe[kt][:, 1:N_TY + 1, :]
                d3 = xo[kt][:, 1:N_TY + 1, :]
                # all DVE: GPSIMD runs these ~5x slower and its SBUF
                # traffic degrades concurrent DVE throughput up to 6x
                nc.vector.tensor_tensor(out=vt[:, 0], in0=d0, in1=d2,
                                        op=mybir.AluOpType.subtract)
                nc.vector.tensor_tensor(out=vt[:, 1], in0=d1, in1=d2,
                                        op=mybir.AluOpType.add)
                nc.vector.tensor_tensor(out=vt[:, 2], in0=d2, in1=d1,
                                        op=mybir.AluOpType.subtract)
                nc.vector.tensor_tensor(out=vt[:, 3], in0=d1, in1=d3,
                                        op=mybir.AluOpType.subtract)
                vtiles[(im, kt)] = vt

            # ---- codebook gather (DVE 12 centroids + GPSIMD 4), add trees ----
            wq = [wq_pool.tile([128, LAB_FREE], BF16, tag=f"wq{kt}",
                               name=f"wq{kt}")
                  for kt in range(N_KT)]
            u12 = [u12_pool.tile([128, N_MT * 2 * 384], BF16, tag=f"u{kt}",
                                 name=f"u12_{kt}")
                   for kt in range(N_KT)]
            t = tbuf_pool.tile([128, N_CENT, CH], BF16, tag="t", name="t")
            s8 = tbuf_pool.tile([128, 8, CH], BF16, tag="s8", name="s8")

            def gather_chunk(mt, kt, marker=False):
                # all on DVE: GPSIMD tensor_scalar(is_equal,mult) measured
                # ~18.6us per pass (12x slower than DVE) — never use it here
                sl = slice(mt * CH, (mt + 1) * CH)
                nc.vector.tensor_copy(out=lab_f[kt][:, sl],
                                      in_=lab_stage.pop((mt, kt))[:])
                for v in range(N_CENT):
                    nc.vector.tensor_scalar(
                        out=t[:, v, :], in0=lab_f[kt][:, sl],
                        scalar1=float(v), scalar2=cent_sb[:, v:v + 1],
                        op0=mybir.AluOpType.is_equal,
                        op1=mybir.AluOpType.mult,
                    )
                if marker:
                    # reads the last eq plane (true RAW) and touches the
                    # region the warm-up matmuls read: the burst launches as
                    # the last eq pass retires, warming the PE exactly while
                    # the tree/U/V ops finish
                    nc.vector.tensor_copy(out=warm_sb[:, 0:1],
                                          in_=t[:, N_CENT - 1, 0:1])
                # add tree, zigzag through consumed slices (same-engine
                # in-order, so WAR overwrites are safe); flat 2-level views
                tf = t[:].rearrange("p a b -> p (a b)")
                sf = s8[:].rearrange("p a b -> p (a b)")
                nc.vector.tensor_tensor(out=sf, in0=tf[:, 0:8 * CH],
                                        in1=tf[:, 8 * CH:16 * CH],
                                        op=mybir.AluOpType.add)
                nc.vector.tensor_tensor(out=tf[:, 0:4 * CH], in0=sf[:, 0:4 * CH],
                                        in1=sf[:, 4 * CH:8 * CH],
                                        op=mybir.AluOpType.add)
                nc.vector.tensor_tensor(out=tf[:, 4 * CH:6 * CH],
                                        in0=tf[:, 0:2 * CH],
                                        in1=tf[:, 2 * CH:4 * CH],
                                        op=mybir.AluOpType.add)
                nc.vector.tensor_tensor(out=wq[kt][:, sl], in0=tf[:, 4 * CH:5 * CH],
                                        in1=tf[:, 5 * CH:6 * CH],
                                        op=mybir.AluOpType.add)

            def u_transform(mt, kt):
                # g_ky plane = wq[kt][:, ((mt*3+ky)*3+0)*128 : +384]
                def g(ky):
                    off = ((mt * 3 + ky) * 3) * 128
                    return wq[kt][:, off:off + 384]
                st = stmp_pool.tile([128, 384], BF16, tag="st",
                                    name=f"st{mt}_{kt}")
                nc.vector.tensor_tensor(out=st[:], in0=g(0), in1=g(2),
                                        op=mybir.AluOpType.add)
                u0 = (mt * 2 + 0) * 384
                u1 = (mt * 2 + 1) * 384
                nc.vector.tensor_tensor(out=u12[kt][:, u0:u0 + 384], in0=st[:],
                                        in1=g(1), op=mybir.AluOpType.add)
                nc.vector.tensor_tensor(out=u12[kt][:, u1:u1 + 384], in0=st[:],
                                        in1=g(1), op=mybir.AluOpType.subtract)

            def lhsT(p, mt, kt, kx):
                if p == 0 or p == 3:
                    ky = 0 if p == 0 else 2
                    off = ((mt * 3 + ky) * 3 + kx) * 128
                    return wq[kt][:, off:off + 128]
                off = ((mt * 2 + (p - 1)) * 3 + kx) * 128
                return u12[kt][:, off:off + 128]

            bias_sb = consts.tile([128, N_MT], F32)

            def conv_group(im, mt, t4):
                ps = [psum_pool.tile([128, NFREE], F32, tag="ps", name="ps")
                      for _ in range(4)]
                ty0 = t4 * TY_T
                for p in range(4):
                    idx = 0
                    for kt in range(N_KT):
                        vt = vtiles[(im, kt)]
                        for kx in range(3):
                            rhs = vt[:, p, ty0:ty0 + TY_T, kx:kx + W]
                            nc.tensor.matmul(ps[p][:], lhsT(p, mt, kt, kx), rhs,
                                             start=(idx == 0), stop=(idx == 5))
                            idx += 1
                # output transform + bias + store
                bias_ap = bias_sb[:, mt:mt + 1]
                m1h = mh_pool.tile([128, NFREE], F32, tag="mh", name="m1h")
                m2h = mh_pool.tile([128, NFREE], F32, tag="mh", name="m2h")
                nc.scalar.activation(out=m1h[:], in_=ps[1][:],
                                     func=mybir.ActivationFunctionType.Copy,
                                     scale=0.5)
                nc.scalar.activation(out=m2h[:], in_=ps[2][:],
                                     func=mybir.ActivationFunctionType.Copy,
                                     scale=0.5)
                aa = asd_pool.tile([128, NFREE], F32, tag="aa", name="aa")
                sd = asd_pool.tile([128, NFREE], F32, tag="sd", name="sd")
                nc.vector.tensor_tensor(out=aa[:], in0=m1h[:], in1=m2h[:],
                                        op=mybir.AluOpType.add)
                nc.gpsimd.tensor_tensor(out=sd[:], in0=m1h[:], in1=m2h[:],
                                        op=mybir.AluOpType.subtract)
                ob = obuf_pool.tile([128, 2 * NFREE], F32, tag="ob", name="ob")
                obv = ob[:].rearrange("p (t a w) -> p t a w", t=TY_T, a=2)
                # out[2ty+0] = (M0 + b) + 0.5(M1'+M2')
                nc.vector.scalar_tensor_tensor(
                    out=obv[:, :, 0, :],
                    in0=ps[0][:].rearrange("p (t w) -> p t w", t=TY_T),
                    scalar=bias_ap, in1=aa[:].rearrange("p (t w) -> p t w", t=TY_T),
                    op0=mybir.AluOpType.add, op1=mybir.AluOpType.add)
                # out[2ty+1] = (0.5(M1'-M2') + b) - M3
                nc.vector.scalar_tensor_tensor(
                    out=obv[:, :, 1, :],
                    in0=sd[:].rearrange("p (t w) -> p t w", t=TY_T),
                    scalar=bias_ap, in1=ps[3][:].rearrange("p (t w) -> p t w", t=TY_T),
                    op0=mybir.AluOpType.add, op1=mybir.AluOpType.subtract)
                # sync queue is free once the startup labels land, and a
                # sync-dispatched store never head-of-line-blocks a compute
                # engine (GPSIMD dispatch stalled its sd ops)
                nc.sync.dma_start(
                    out=out[im, mt][:, t4 * 2 * NFREE:(t4 + 1) * 2 * NFREE],
                    in_=ob[:])

            # ---- emission schedule ----
            load_labels(0)
            for kt in range(N_KT):
                load_x(0, kt)
            for mt in range(N_MT):
                nc.sync.dma_start(out=bias_sb[:, mt:mt + 1], in_=bias_in[mt, :])

            # Staging is emitted in dependency-correct order (V(im) before
            # load_x(im+1) overwrites xe/xo); emission index doubles as the
            # scheduler priority, so staging naturally outranks the later
            # eviction work without explicit priority overrides.
            gather_chunk(0, 0)
            u_transform(0, 0)
            v_transform(0, 0)
            gather_chunk(0, 1, marker=True)
            u_transform(0, 1)
            v_transform(0, 1)
            # ---- PE HAM warm-up burst: dummy matmuls, result discarded ----
            warm_ps = psum_pool.tile([128, 512], F32, tag="ps")
            for _ in range(N_WARM):
                nc.tensor.matmul(warm_ps[:], warm_sb[:, :128], warm_sb[:],
                                 start=True, stop=True)
            load_labels(1)
            for kt in range(N_KT):
                load_x(1, kt)
            for kt in range(N_KT):
                v_transform(1, kt)

            for i, (im, mt) in enumerate(PAIR_ORDER):
                for t4 in range(N_T4):
                    conv_group(im, mt, t4)
                if i == 0:            # after pair (0, 0)
                    # mt=1 gather spread over two pair boundaries: evictions
                    # of the running pairs outrank it on DVE, and it still
                    # finishes well before pair (0, 1) needs U(1)
                    gather_chunk(1, 0)
                    u_transform(1, 0)
                    for kt in range(N_KT):
                        load_x(2, kt)
                    for kt in range(N_KT):
                        v_transform(2, kt)   # fresh ring buffer: no WAR wait
                elif i == 1:          # after pair (1, 0)
                    gather_chunk(1, 1)
                    u_transform(1, 1)
                elif i == 3:          # after pair (0, 1)
                    for kt in range(N_KT):
                        load_x(3, kt)
                    for kt in range(N_KT):
                        v_transform(3, kt)

    _split_multi_waits(nc)
    return nc


_NC_CACHE = None


def _get_nc():
    global _NC_CACHE
    if _NC_CACHE is None:
        _NC_CACHE = build_program()
    return _NC_CACHE


def make_in_maps(x, centroids, labels, bias):
    """Shard full inputs into 8 per-core input maps."""
    x = np.ascontiguousarray(x, dtype=np.float32)
    centroids = np.ascontiguousarray(centroids, dtype=np.float32)
    labels = np.ascontiguousarray(labels, dtype=np.int32)
    bias = np.ascontiguousarray(bias, dtype=np.float32)

    in_maps = []
    for c in range(8):
        b, g = c // 2, c % 2
        xs = x[4 * b: 4 * b + 4].reshape(N_IMG_PER_CORE, N_KT, 128, HW)
        lg = labels[256 * g: 256 * g + 256]          # [256o, 256c, 3, 3]
        lg = lg.reshape(N_MT, 128, N_KT, 128, 3, 3)  # [mt, oo, kt, cc, ky, kx]
        lg = lg.transpose(2, 3, 0, 4, 5, 1)          # [kt, cc, mt, ky, kx, oo]
        lg = np.ascontiguousarray(lg).reshape(N_KT, 128, LAB_FREE)
        bg = bias[g].reshape(N_MT, 128)
        in_maps.append({
            "x": np.ascontiguousarray(xs),
            "labels": lg,
            "centroids": centroids,
            "bias": np.ascontiguousarray(bg),
        })
    return in_maps


def run(x, centroids, labels, bias, trace=False, trace_cores=None):
    nc = _get_nc()
    in_maps = make_in_maps(x, centroids, labels, bias)
    res = run_bass_kernel_spmd(nc, in_maps, list(range(8)), trace=trace,
                               trace_cores=trace_cores)
    out0 = np.empty((16, 256, H, W), dtype=np.float32)
    out1 = np.empty((16, 256, H, W), dtype=np.float32)
    for c in range(8):
        b, g = c // 2, c % 2
        o = res.results[c]["out"].reshape(N_IMG_PER_CORE, 256, H, W)
        (out0 if g == 0 else out1)[4 * b: 4 * b + 4] = o
    return (out0, out1), res


def kernel(x, centroids, labels, bias):
    (out0, out1), _ = run(x, centroids, labels, bias, trace=False)
    return (out0, out1)
